# revision 6
# baseline (speedup 1.0000x reference)
"""Trainium2 Bass kernel for the CompositionalAgent cell (DND retrieval + LSTM gates + A2C head).

Distribution over 8 NeuronCores:
  * DND key/value tables (50000 x {6,1} keys, 50000 x 2048 values) are sharded
    row-wise: 6250 memory slots per core.  Each core computes local
    exp(cos-sim) weights and a local softmax-numerator-weighted partial value
    sum; a single ReduceScatter combines (numerator, denominator) so that core
    k receives exactly its 256-wide hidden slice of both tables' numerators
    plus the global denominators.
  * Gate GEMVs (W_i2h / W_h2h, 6H outputs) are tensor-parallel: core k owns
    hidden slice [256k, 256k+256) of every gate.
  * The A2C head GEMV (W_ih) is tensor-parallel along the contraction dim;
    one AllReduce combines the 2048 partial pre-activations, after which each
    core redundantly computes relu / actor / critic / softmax.
  * h_out / c_out are emitted as per-core 256-slices and concatenated on host.
    The DND memory write (row replacement at write_idx) is pure data movement
    and is assembled on host from c_out.

The same Bass graph runs SPMD on all 8 cores; per-core behaviour comes only
from per-core input shards (the ReduceScatter block layout delivers each core
its own slice without any core-id-dependent addressing).
"""

import os

import numpy as np

os.environ.setdefault("MYCRO_LOCAL_CACHE", "1")

import concourse.bass as bass
import concourse.mybir as mybir
from concourse import bacc, bass_utils, tile

F32 = mybir.dt.float32
AF = mybir.ActivationFunctionType

# Model dims (hardcoded per problem spec).
H = 2048
OUT = 8
DICT = 50000
FDIM = 6
RDIM = 1
IN = 16
NG = 5
EPS = 1e-8

NCORES = 8
ROWS = DICT // NCORES          # 6250 memory slots per core
CH = (ROWS + 127) // 128       # 49 slot-chunks of 128
PADK = CH * 128                # 6272 (keys padded with zeros)
LASTK = ROWS - (CH - 1) * 128  # 106 real slots in the last chunk
PADCNT = float(PADK - ROWS)    # 22 zero-key pad slots, each contributes exp(0)=1
HSL = H // NCORES              # 256 hidden slice per core
GSL = (NG + 1) * HSL           # 1536 local gate-preactivation rows
BLK = 520                      # RS block: 256 n_f + 256 n_r + d_f + d_r + 6 pad

_CACHE = {}
LAST_RESULT = None
LAST_EXEC_WALL_S = None


def _emit(nc, tc, t):
    """Emit the SPMD per-core graph. `t` maps tensor name -> bass.AP."""
    with (
        tc.tile_pool(name="small", bufs=1) as small,
        tc.tile_pool(name="vpool", bufs=3) as vpool,
        tc.tile_pool(name="whpool", bufs=2) as whpool,
        tc.tile_pool(name="wipool", bufs=2) as wipool,
        tc.tile_pool(name="sqpool", bufs=1) as sqpool,
        tc.tile_pool(name="psacc", bufs=4, space="PSUM") as psacc,
        tc.tile_pool(name="psdn", bufs=3, space="PSUM") as psdn,
        tc.tile_pool(name="pssm", bufs=1, space="PSUM") as pssm,
        tc.tile_pool(name="dpool", bufs=1, space="DRAM") as dpool,
    ):
        # ---- DRAM bounce buffers for collectives + transpose scratch ----
        cc1_in = dpool.tile([NCORES, BLK], F32)
        cc1_out = dpool.tile([1, BLK], F32)
        cc2_in = dpool.tile([1, H], F32)
        cc2_out = dpool.tile([1, H], F32)
        ht_scr = dpool.tile([1, HSL], F32)

        # ---- small persistent SBUF tiles ----
        x_sb = small.tile([1, IN], F32)
        nc.sync.dma_start(x_sb, t["x_row"])
        xcol_sb = small.tile([IN, 1], F32)
        nc.sync.dma_start(xcol_sb, t["x_row"])
        qf_col = small.tile([FDIM, 1], F32)
        nc.sync.dma_start(qf_col, t["x_row"][0:1, 0:FDIM])

        h_sb = small.tile([128, H // 128], F32)
        nc.sync.dma_start(h_sb, t["h_lay"])
        c_sb = small.tile([1, HSL], F32)
        nc.sync.dma_start(c_sb, t["c_sl"])
        b1_sb = small.tile([1, GSL], F32)
        nc.sync.dma_start(b1_sb, t["b1"])
        b2_sb = small.tile([1, GSL], F32)
        nc.sync.dma_start(b2_sb, t["b2"])
        bih_sb = small.tile([128, H // 128], F32)
        nc.sync.dma_start(bih_sb, t["bih_lay"])
        bact_sb = small.tile([1, OUT + 1], F32)
        nc.sync.dma_start(bact_sb, t["bact"])
        wi2h_sb = small.tile([IN, GSL], F32)
        nc.sync.dma_start(wi2h_sb, t["wi2h_t"])
        # W_actor/W_critic packed: [128, 16*9], col t*9+j = row (t*128+p) of [2048, 9]
        wact_sb = small.tile([128, (H // 128) * (OUT + 1)], F32)
        nc.sync.dma_start(
            wact_sb, t["wact_t"].rearrange("(a p) j -> p a j", p=128)
        )

        keysf_sb = small.tile([FDIM, PADK], F32)
        nc.sync.dma_start(keysf_sb, t["fkeys_t"])
        rk_sb = small.tile([128, CH], F32)
        nc.sync.dma_start(rk_sb, t["rkeys_lay"])

        onesf = small.tile([FDIM, 1], F32)
        nc.vector.memset(onesf, 1.0)
        ones128 = small.tile([128, 1], F32)
        nc.vector.memset(ones128, 1.0)
        ones8 = small.tile([1, NCORES], F32)
        nc.vector.memset(ones8, 1.0)
        ddpad = small.tile([1, NCORES], F32)
        nc.vector.memset(ddpad, 0.0)

        # ---- query norms: invq = 1 / (||q|| + eps) ----
        qsq = small.tile([1, IN], F32)
        nc.scalar.square(qsq, x_sb)
        invq = {}
        invq_bc = {}
        for name, lo, hi in (("f", 0, FDIM), ("r", FDIM, FDIM + RDIM)):
            qss = small.tile([1, 1], F32, name=f"qss_{name}")
            nc.vector.reduce_sum(qss, qsq[0:1, lo:hi], axis=mybir.AxisListType.X)
            qn = small.tile([1, 1], F32, name=f"qn_{name}")
            nc.scalar.sqrt(qn, qss)
            qne = small.tile([1, 1], F32, name=f"qne_{name}")
            nc.vector.tensor_scalar_add(qne, qn, EPS)
            iq = small.tile([1, 1], F32, name=f"invq_{name}")
            nc.vector.reciprocal(iq, qne)
            invq[name] = iq
            bc = small.tile([128, 1], F32, name=f"invq_bc_{name}")
            nc.gpsimd.partition_broadcast(bc, iq)
            invq_bc[name] = bc

        def value_sum(name, e_all, vals_dram):
            """n = sum_i e_i * vals[i, :] into 4 psum accs, then to SBUF + RS block."""
            accs = []
            for g in range(H // 512):
                a = psacc.tile([1, 512], F32, tag="acc", name=f"nacc_{name}_{g}")
                accs.append(a)
            for c in range(CH):
                rows = 128 if c < CH - 1 else LASTK
                vt = vpool.tile([128, H], F32, tag="vt", name=f"vt_{name}_{c}")
                nc.sync.dma_start(
                    vt[0:rows, :], vals_dram[c * 128:c * 128 + rows, :]
                )
                for g in range(H // 512):
                    nc.tensor.matmul(
                        accs[g],
                        e_all[0:rows, c:c + 1],
                        vt[0:rows, g * 512:(g + 1) * 512],
                        start=(c == 0),
                        stop=(c == CH - 1),
                    )
            nvec = small.tile([1, H], F32, name=f"nvec_{name}")
            for g in range(H // 512):
                nc.scalar.copy(nvec[0:1, g * 512:(g + 1) * 512], accs[g])
            return nvec

        def denom(name, e_all, dd_col):
            """ddpad[0, dd_col] = sum(e_all) - PADCNT."""
            esum = small.tile([128, 1], F32, name=f"esum_{name}")
            nc.vector.reduce_sum(esum, e_all, axis=mybir.AxisListType.X)
            dpart = pssm.tile([1, 1], F32, tag="sm", name=f"dpart_{name}")
            nc.tensor.matmul(dpart, esum, ones128, start=True, stop=True)
            nc.vector.tensor_scalar_add(ddpad[0:1, dd_col:dd_col + 1], dpart, -PADCNT)

        # =========== DND f-table: sims on PE (FDIM=6) ===========
        sqf_sb = sqpool.tile([FDIM, PADK], F32, tag="sq")
        nc.scalar.square(sqf_sb, keysf_sb)
        dn = psdn.tile([128, 2 * CH], F32, tag="dnacc", name="dn_f")
        for c in range(CH):
            ks = keysf_sb[:, c * 128:(c + 1) * 128]
            nc.tensor.matmul(dn[:, 2 * c:2 * c + 1], ks, qf_col, start=True, stop=True)
            sqs = sqf_sb[:, c * 128:(c + 1) * 128]
            nc.tensor.matmul(dn[:, 2 * c + 1:2 * c + 2], sqs, onesf, start=True, stop=True)
        dn3 = dn.rearrange("p (c k) -> p c k", k=2)
        normf = small.tile([128, CH], F32)
        nc.scalar.sqrt(normf, dn3[:, :, 1])
        normfe = small.tile([128, CH], F32)
        nc.vector.tensor_scalar_add(normfe, normf, EPS)
        invf = small.tile([128, CH], F32)
        nc.vector.reciprocal(invf, normfe)
        simf = small.tile([128, CH], F32)
        nc.vector.tensor_mul(simf, dn3[:, :, 0], invf)
        e_f = small.tile([128, CH], F32)
        nc.scalar.activation(e_f, simf, AF.Exp, scale=invq_bc["f"])
        denom("f", e_f, 0)
        nvec_f = value_sum("f", e_f, t["fvals"])
        nc.sync.dma_start(
            cc1_in[:, 0:HSL], nvec_f.rearrange("o (b u) -> o b u", b=NCORES)
        )

        # =========== gate GEMV (tensor parallel along 6H) ===========
        # preact[m] = sum_k W_h2h_t[k, m] * h[k] + sum_j W_i2h_t[j, m] * x[j]
        pre_g = []
        for g in range(GSL // 512):
            pg = psdn.tile([1, 512], F32, tag="dnacc", name=f"pre_{g}")
            pre_g.append(pg)
        for k0 in range(H // 128):
            wht = whpool.tile([128, GSL], F32, tag="wh", name=f"wht_{k0}")
            nc.sync.dma_start(wht, t["wh2h_t"][k0 * 128:(k0 + 1) * 128, :])
            for g in range(GSL // 512):
                nc.tensor.matmul(
                    pre_g[g],
                    h_sb[:, k0:k0 + 1],
                    wht[:, g * 512:(g + 1) * 512],
                    start=(k0 == 0),
                    stop=False,
                )
        for g in range(GSL // 512):
            nc.tensor.matmul(
                pre_g[g],
                xcol_sb,
                wi2h_sb[:, g * 512:(g + 1) * 512],
                start=False,
                stop=True,
            )
        pre_sb = small.tile([1, GSL], F32)
        for g in range(GSL // 512):
            nc.vector.tensor_add(
                pre_sb[0:1, g * 512:(g + 1) * 512],
                pre_g[g],
                b1_sb[0:1, g * 512:(g + 1) * 512],
            )
        pre2_sb = small.tile([1, GSL], F32)
        nc.vector.tensor_add(pre2_sb, pre_sb, b2_sb)
        gates_sb = small.tile([1, NG * HSL], F32)
        nc.scalar.activation(gates_sb, pre2_sb[0:1, 0:NG * HSL], AF.Sigmoid)
        cnew_sb = small.tile([1, HSL], F32)
        nc.scalar.activation(cnew_sb, pre2_sb[0:1, NG * HSL:GSL], AF.Tanh)

        # =========== DND r-table: sims on DVE/ACT (RDIM=1, slot-major) ===========
        # sim_i = k_i * q_r / ((|k_i|+eps)(|q_r|+eps)); e = exp(sim)
        absr = small.tile([128, CH], F32)
        nc.scalar.activation(absr, rk_sb, AF.Abs)
        absre = small.tile([128, CH], F32)
        nc.vector.tensor_scalar_add(absre, absr, EPS)
        invr = small.tile([128, CH], F32)
        nc.vector.reciprocal(invr, absre)
        simr = small.tile([128, CH], F32)
        nc.vector.tensor_mul(simr, rk_sb, invr)
        # scale = q_r * invq_r, broadcast across partitions
        qr_sc = small.tile([1, 1], F32)
        nc.scalar.copy(qr_sc, x_sb[0:1, FDIM:FDIM + 1])
        qrs = small.tile([1, 1], F32)
        nc.vector.tensor_mul(qrs, qr_sc, invq["r"])
        qrs_bc = small.tile([128, 1], F32)
        nc.gpsimd.partition_broadcast(qrs_bc, qrs)
        e_r = small.tile([128, CH], F32)
        nc.scalar.activation(e_r, simr, AF.Exp, scale=qrs_bc)
        denom("r", e_r, 1)
        nvec_r = value_sum("r", e_r, t["rvals"])
        nc.sync.dma_start(
            cc1_in[:, HSL:2 * HSL], nvec_r.rearrange("o (b u) -> o b u", b=NCORES)
        )

        # replicate [d_f, d_r, 0...] into all 8 RS blocks via a K=1 matmul
        ddrep = pssm.tile([NCORES, NCORES], F32, tag="sm")
        nc.tensor.matmul(ddrep, ones8, ddpad, start=True, stop=True)
        ddrep_sb = small.tile([NCORES, NCORES], F32)
        nc.vector.tensor_copy(ddrep_sb, ddrep)
        nc.sync.dma_start(cc1_in[:, 2 * HSL:2 * HSL + NCORES], ddrep_sb)

        # ---- collective 1: ReduceScatter -> this core's slice ----
        nc.gpsimd.collective_compute(
            "ReduceScatter",
            mybir.AluOpType.add,
            replica_groups=[list(range(NCORES))],
            ins=[cc1_in.opt()],
            outs=[cc1_out.opt()],
        )

        mf_raw = small.tile([1, HSL], F32)
        nc.sync.dma_start(mf_raw, cc1_out[0:1, 0:HSL])
        mr_raw = small.tile([1, HSL], F32)
        nc.sync.dma_start(mr_raw, cc1_out[0:1, HSL:2 * HSL])
        dsum = small.tile([1, 2], F32)
        nc.sync.dma_start(dsum, cc1_out[0:1, 2 * HSL:2 * HSL + 2])
        dinv = small.tile([1, 2], F32)
        nc.vector.reciprocal(dinv, dsum)
        memfun = small.tile([1, HSL], F32)
        tmpf = small.tile([1, HSL], F32)
        nc.vector.tensor_scalar_mul(tmpf, mf_raw, dinv[0:1, 0:1])
        nc.scalar.activation(memfun, tmpf, AF.Tanh)
        memrul = small.tile([1, HSL], F32)
        tmpr = small.tile([1, HSL], F32)
        nc.vector.tensor_scalar_mul(tmpr, mr_raw, dinv[0:1, 1:2])
        nc.scalar.activation(memrul, tmpr, AF.Tanh)

        # ---- c_t = f*c + i*cnew + fun*memfun + rul*memrul ; h_t = o*tanh(c_t) ----
        f_t = gates_sb[0:1, 0 * HSL:1 * HSL]
        i_t = gates_sb[0:1, 1 * HSL:2 * HSL]
        o_t = gates_sb[0:1, 2 * HSL:3 * HSL]
        fun_t = gates_sb[0:1, 3 * HSL:4 * HSL]
        rul_t = gates_sb[0:1, 4 * HSL:5 * HSL]

        ct_a = small.tile([1, HSL], F32)
        nc.vector.tensor_mul(ct_a, f_t, c_sb)
        ct_b = small.tile([1, HSL], F32)
        nc.vector.tensor_mul(ct_b, i_t, cnew_sb)
        ct_c = small.tile([1, HSL], F32)
        nc.vector.tensor_add(ct_c, ct_a, ct_b)
        ct_d = small.tile([1, HSL], F32)
        nc.vector.tensor_mul(ct_d, fun_t, memfun)
        ct_e = small.tile([1, HSL], F32)
        nc.vector.tensor_add(ct_e, ct_c, ct_d)
        ct_f = small.tile([1, HSL], F32)
        nc.vector.tensor_mul(ct_f, rul_t, memrul)
        ct = small.tile([1, HSL], F32)
        nc.vector.tensor_add(ct, ct_e, ct_f)

        tct = small.tile([1, HSL], F32)
        nc.scalar.activation(tct, ct, AF.Tanh)
        ht = small.tile([1, HSL], F32)
        nc.vector.tensor_mul(ht, o_t, tct)

        nc.sync.dma_start(t["o_c"], ct)
        nc.sync.dma_start(t["o_h"], ht)

        # ---- h_t slice -> partition-major [128, 2] via DRAM bounce ----
        nc.sync.dma_start(ht_scr, ht)
        htcol = small.tile([128, HSL // 128], F32)
        nc.sync.dma_start(htcol, ht_scr.rearrange("o (a p) -> (o p) a", p=128))

        # ---- z partial = W_ih[:, slice] @ h_t[slice] ----
        zacc = []
        for g in range(H // 512):
            z = psacc.tile([1, 512], F32, tag="acc", name=f"zacc_{g}")
            zacc.append(z)
        for kc in range(HSL // 128):
            wt2 = wipool.tile([128, H], F32, tag="wih", name=f"wiht_{kc}")
            nc.sync.dma_start(wt2, t["wih_t"][kc * 128:(kc + 1) * 128, :])
            for g in range(H // 512):
                nc.tensor.matmul(
                    zacc[g],
                    htcol[:, kc:kc + 1],
                    wt2[:, g * 512:(g + 1) * 512],
                    start=(kc == 0),
                    stop=(kc == HSL // 128 - 1),
                )
        zsb = small.tile([1, H], F32)
        for g in range(H // 512):
            nc.scalar.copy(zsb[0:1, g * 512:(g + 1) * 512], zacc[g])
        nc.sync.dma_start(cc2_in, zsb)

        # ---- collective 2: AllReduce of z ----
        nc.gpsimd.collective_compute(
            "AllReduce",
            mybir.AluOpType.add,
            replica_groups=[list(range(NCORES))],
            ins=[cc2_in.opt()],
            outs=[cc2_out.opt()],
        )

        zfull = small.tile([128, H // 128], F32)
        nc.sync.dma_start(zfull, cc2_out.rearrange("o (a p) -> (o p) a", p=128))
        zb = small.tile([128, H // 128], F32)
        nc.vector.tensor_add(zb, zfull, bih_sb)
        ha = small.tile([128, H // 128], F32)
        nc.scalar.activation(ha, zb, AF.Relu)

        # ---- actor/critic: [logits | v] = [W_actor; W_critic] @ ha ----
        logits = pssm.tile([1, OUT + 1], F32, tag="sm")
        for a in range(H // 128):
            nc.tensor.matmul(
                logits,
                ha[:, a:a + 1],
                wact_sb[:, a * (OUT + 1):(a + 1) * (OUT + 1)],
                start=(a == 0),
                stop=(a == H // 128 - 1),
            )
        lsb = small.tile([1, OUT + 1], F32)
        nc.vector.tensor_add(lsb, logits, bact_sb)
        nc.sync.dma_start(t["o_v"], lsb[0:1, OUT:OUT + 1])

        # softmax over the 8 logits
        mx = small.tile([1, 1], F32)
        nc.vector.reduce_max(mx, lsb[0:1, 0:OUT], axis=mybir.AxisListType.X)
        sm_in = small.tile([1, OUT], F32)
        nc.vector.tensor_scalar_sub(sm_in, lsb[0:1, 0:OUT], mx)
        pexp = small.tile([1, OUT], F32)
        nc.scalar.activation(pexp, sm_in, AF.Exp)
        se = small.tile([1, 1], F32)
        nc.vector.reduce_sum(se, pexp, axis=mybir.AxisListType.X)
        sinv = small.tile([1, 1], F32)
        nc.vector.reciprocal(sinv, se)
        pi = small.tile([1, OUT], F32)
        nc.vector.tensor_scalar_mul(pi, pexp, sinv)
        nc.sync.dma_start(t["o_pi"], pi)


def _build():
    if "nc" in _CACHE:
        return _CACHE["nc"]
    nc = bacc.Bacc(
        "TRN2",
        target_bir_lowering=False,
        debug=False,
        enable_asserts=False,
        num_devices=NCORES,
    )
    t = {}

    def inp(name, shape):
        t[name] = nc.dram_tensor(name, list(shape), F32, kind="ExternalInput").ap()

    def outp(name, shape):
        t[name] = nc.dram_tensor(name, list(shape), F32, kind="ExternalOutput").ap()

    inp("x_row", (1, IN))
    inp("h_lay", (128, H // 128))
    inp("c_sl", (1, HSL))
    inp("wi2h_t", (IN, GSL))
    inp("wh2h_t", (H, GSL))
    inp("b1", (1, GSL))
    inp("b2", (1, GSL))
    inp("fkeys_t", (FDIM, PADK))
    inp("rkeys_lay", (128, CH))
    inp("fvals", (ROWS, H))
    inp("rvals", (ROWS, H))
    inp("wih_t", (HSL, H))
    inp("bih_lay", (128, H // 128))
    inp("wact_t", (H, OUT + 1))
    inp("bact", (1, OUT + 1))
    outp("o_pi", (1, OUT))
    outp("o_v", (1, 1))
    outp("o_h", (1, HSL))
    outp("o_c", (1, HSL))

    with tile.TileContext(nc) as tc:
        _emit(nc, tc, t)
    nc.compile()
    _CACHE["nc"] = nc
    return nc


def _shard_inputs(inputs):
    f32c = lambda a: np.ascontiguousarray(np.asarray(a, dtype=np.float32))
    x_t = f32c(inputs["x_t"]).reshape(1, IN)
    h = f32c(inputs["h"]).reshape(H)
    c = f32c(inputs["c"]).reshape(H)
    W_i2h = f32c(inputs["W_i2h"])
    b_i2h = f32c(inputs["b_i2h"]).reshape(-1)
    W_h2h = f32c(inputs["W_h2h"])
    b_h2h = f32c(inputs["b_h2h"]).reshape(-1)
    f_keys = f32c(inputs["f_keys"])
    f_vals = f32c(inputs["f_vals"])
    r_keys = f32c(inputs["r_keys"])
    r_vals = f32c(inputs["r_vals"])
    W_ih = f32c(inputs["W_ih"])
    b_ih = f32c(inputs["b_ih"]).reshape(-1)
    W_actor = f32c(inputs["W_actor"])
    b_actor = f32c(inputs["b_actor"]).reshape(-1)
    W_critic = f32c(inputs["W_critic"]).reshape(1, H)
    b_critic = f32c(inputs["b_critic"]).reshape(-1)

    h_lay = np.ascontiguousarray(h.reshape(H // 128, 128).T)
    bih_lay = np.ascontiguousarray(b_ih.reshape(H // 128, 128).T)
    wact_t = np.ascontiguousarray(np.concatenate([W_actor, W_critic], axis=0).T)
    bact = np.ascontiguousarray(np.concatenate([b_actor, b_critic]).reshape(1, OUT + 1))

    wh2h_g = W_h2h.reshape(NG + 1, NCORES, HSL, H)
    wi2h_g = W_i2h.reshape(NG + 1, NCORES, HSL, IN)
    b1_g = b_i2h.reshape(NG + 1, NCORES, HSL)
    b2_g = b_h2h.reshape(NG + 1, NCORES, HSL)

    in_maps = []
    for k in range(NCORES):
        sl = slice(k * ROWS, (k + 1) * ROWS)
        fkt = np.zeros((FDIM, PADK), np.float32)
        fkt[:, :ROWS] = f_keys[sl].T
        rk_pad = np.zeros(PADK, np.float32)
        rk_pad[:ROWS] = r_keys[sl, 0]
        rk_lay = np.ascontiguousarray(rk_pad.reshape(CH, 128).T)
        wh2h_t = np.ascontiguousarray(wh2h_g[:, k].reshape(GSL, H).T)
        wi2h_t = np.ascontiguousarray(wi2h_g[:, k].reshape(GSL, IN).T)
        wih_t = np.ascontiguousarray(W_ih[:, k * HSL:(k + 1) * HSL].T)
        in_maps.append(
            {
                "x_row": x_t,
                "h_lay": h_lay,
                "c_sl": np.ascontiguousarray(c[k * HSL:(k + 1) * HSL]).reshape(1, HSL),
                "wi2h_t": wi2h_t,
                "wh2h_t": wh2h_t,
                "b1": np.ascontiguousarray(b1_g[:, k].reshape(1, GSL)),
                "b2": np.ascontiguousarray(b2_g[:, k].reshape(1, GSL)),
                "fkeys_t": fkt,
                "rkeys_lay": rk_lay,
                "fvals": np.ascontiguousarray(f_vals[sl]),
                "rvals": np.ascontiguousarray(r_vals[sl]),
                "wih_t": wih_t,
                "bih_lay": bih_lay,
                "wact_t": wact_t,
                "bact": bact,
            }
        )
    return in_maps


def kernel(**inputs):
    global LAST_RESULT, LAST_EXEC_WALL_S
    import time

    nc = _build()
    in_maps = _shard_inputs(inputs)

    trace = os.environ.get("KERNEL_TRACE") == "1"
    t0 = time.monotonic()
    res = bass_utils.run_bass_kernel_spmd(
        nc, in_maps, core_ids=list(range(NCORES)), trace=trace
    )
    LAST_EXEC_WALL_S = time.monotonic() - t0
    LAST_RESULT = res
    r = res.results

    pi = np.asarray(r[0]["o_pi"], np.float32).reshape(1, OUT)
    v = np.asarray(r[0]["o_v"], np.float32).reshape(1, 1)
    h_out = np.concatenate(
        [np.asarray(r[k]["o_h"], np.float32).reshape(HSL) for k in range(NCORES)]
    ).reshape(1, 1, H)
    c_out = np.concatenate(
        [np.asarray(r[k]["o_c"], np.float32).reshape(HSL) for k in range(NCORES)]
    ).reshape(1, 1, H)

    # DND memory write (pure data movement) assembled on host from device c_t.
    x_t = np.asarray(inputs["x_t"], np.float32).reshape(1, IN)
    widx = int(np.asarray(inputs["write_idx"]))
    q_f = x_t[0, 0:FDIM]
    q_r = x_t[0, FDIM:FDIM + RDIM]
    c_row = c_out[0, 0]
    c_ft = c_row.copy()
    c_ft[H // 2:] = 0.0
    c_rt = c_row.copy()
    c_rt[:H // 2] = 0.0
    new_f_keys = np.asarray(inputs["f_keys"], np.float32).copy()
    new_f_keys[widx] = q_f
    new_f_vals = np.asarray(inputs["f_vals"], np.float32).copy()
    new_f_vals[widx] = c_ft
    new_r_keys = np.asarray(inputs["r_keys"], np.float32).copy()
    new_r_keys[widx] = q_r
    new_r_vals = np.asarray(inputs["r_vals"], np.float32).copy()
    new_r_vals[widx] = c_rt

    return (pi, v, h_out, c_out, new_f_keys, new_f_vals, new_r_keys, new_r_vals)
